# revision 2
# baseline (speedup 1.0000x reference)
"""RBF (Gaussian) kernel matrix on 8 TRN2 NeuronCores.

out[i, j] = exp(-gamma * ||x_i - y_j||^2),  x: [8192, 64], y: [8192, 64].

Strategy: shard rows of x across 8 cores (each computes a [1024, 8192]
tile), replicate y.  The squared distance is produced directly by matmul
via augmented vectors:

    u_i = [-2*x_i, |x_i|^2, 1]   (K = 66)
    v_j = [   y_j,       1, |y_j|^2]

so  u_i . v_j = |x_i|^2 + |y_j|^2 - 2 x_i.y_j = dist2[i, j].  PSUM then
holds dist2 directly and one ScalarE activation computes
exp(-gamma * dist2) per [128, 512] tile — no vector-engine work at all.
Output is staged into [128, 8192] SBUF strips so each store is a single
4 MB DMA (near peak HBM write bandwidth).

MODE selects the matmul precision strategy:
  "f32"    — native fp32 matmul (4 cycles/row on the PE).
  "f32r"   — single-pass fp32 (tf32-like, 1 cycle/row), reduced precision.
  "bf16x3" — split each operand a = hi + lo in bf16 and accumulate
             hi*hi + hi*lo + lo*hi in PSUM (3 bf16 matmuls, ~18-bit
             effective mantissa, 3 cycles/row).
"""

import numpy as np

N_X, N_Y, D = 8192, 8192, 64
N_CORES = 8
N_PER = N_X // N_CORES  # rows of x per core
K_AUG = D + 2  # 66

MODE = "f32"

# Filled by kernel() with the BassKernelResults of the last run
# (test.py reads exec_time_ns from here when BASS_TRACE=1).
LAST_RESULTS = None

_BUILD_CACHE = {}


def _build(gamma: float, n_per: int, m_tot: int, mode: str):
    """Build + compile the single-core Bass program (same on all cores)."""
    import concourse.bacc as bacc
    import concourse.mybir as mybir
    import concourse.tile as tile

    key = (gamma, n_per, m_tot, mode)
    if key in _BUILD_CACHE:
        return _BUILD_CACHE[key]

    dt = mybir.dt
    in_dt = {"f32": dt.float32, "f32r": dt.float32r, "bf16x3": dt.bfloat16}[mode]
    nsplit = 2 if mode == "bf16x3" else 1

    nc = bacc.Bacc("TRN2", target_bir_lowering=False, debug=False)
    ut_d = [
        nc.dram_tensor(f"ut{i}", [K_AUG, n_per], in_dt, kind="ExternalInput").ap()
        for i in range(nsplit)
    ]
    vt_d = [
        nc.dram_tensor(f"vt{i}", [K_AUG, m_tot], in_dt, kind="ExternalInput").ap()
        for i in range(nsplit)
    ]
    out_d = nc.dram_tensor("out", [n_per, m_tot], dt.float32, kind="ExternalOutput").ap()

    MB = n_per // 128  # M-blocks (output partition tiles)
    NB = m_tot // 512  # N-blocks (PSUM-bank-sized column tiles)

    with tile.TileContext(nc) as tc:
        with (
            tc.tile_pool(name="const", bufs=1) as cpool,
            tc.tile_pool(name="psum", bufs=4, space="PSUM") as psum_pool,
            tc.tile_pool(name="strip", bufs=2) as strip_pool,
        ):
            ut_s = []
            vt_s = []
            for i in range(nsplit):
                u = cpool.tile([K_AUG, n_per], in_dt, tag=f"ut{i}")
                nc.sync.dma_start(u[:], ut_d[i][:])
                ut_s.append(u)
                v = cpool.tile([K_AUG, m_tot], in_dt, tag=f"vt{i}")
                nc.sync.dma_start(v[:], vt_d[i][:])
                vt_s.append(v)

            for m in range(MB):
                strip = strip_pool.tile([128, m_tot], dt.float32)
                msl = slice(m * 128, (m + 1) * 128)
                for n in range(NB):
                    nsl = slice(n * 512, (n + 1) * 512)
                    ps = psum_pool.tile([128, 512], dt.float32)
                    if nsplit == 1:
                        nc.tensor.matmul(ps[:], ut_s[0][:, msl], vt_s[0][:, nsl])
                    else:
                        # hi*hi + hi*lo (same weights) + lo*hi, accumulated
                        nc.tensor.matmul(
                            ps[:], ut_s[0][:, msl], vt_s[0][:, nsl],
                            start=True, stop=False,
                        )
                        nc.tensor.matmul(
                            ps[:], ut_s[0][:, msl], vt_s[1][:, nsl],
                            start=False, stop=False,
                        )
                        nc.tensor.matmul(
                            ps[:], ut_s[1][:, msl], vt_s[0][:, nsl],
                            start=False, stop=True,
                        )
                    nc.scalar.activation(
                        strip[:, nsl],
                        ps[:],
                        mybir.ActivationFunctionType.Exp,
                        scale=-gamma,
                    )
                nc.sync.dma_start(out_d[msl, :], strip[:])

    nc.compile()
    _BUILD_CACHE[key] = nc
    return nc


def _augment(x: np.ndarray, y: np.ndarray):
    """Host-side prep: build transposed augmented operands (O(N*D) work)."""
    x = np.asarray(x, dtype=np.float32)
    y = np.asarray(y, dtype=np.float32)
    x2 = np.einsum("nd,nd->n", x, x).astype(np.float32)
    y2 = np.einsum("nd,nd->n", y, y).astype(np.float32)

    ut = np.empty((K_AUG, x.shape[0]), dtype=np.float32)
    ut[:D] = (-2.0 * x).T
    ut[D] = x2
    ut[D + 1] = 1.0

    vt = np.empty((K_AUG, y.shape[0]), dtype=np.float32)
    vt[:D] = y.T
    vt[D] = 1.0
    vt[D + 1] = y2
    return ut, vt


def _split_bf16(a32: np.ndarray):
    import ml_dtypes

    hi = a32.astype(ml_dtypes.bfloat16)
    lo = (a32 - hi.astype(np.float32)).astype(ml_dtypes.bfloat16)
    return [hi, lo]


def kernel(x: np.ndarray, y: np.ndarray, gamma: np.ndarray) -> np.ndarray:
    global LAST_RESULTS
    from concourse.bass_utils import run_bass_kernel_spmd

    gamma_f = float(np.asarray(gamma).reshape(()))
    ut, vt = _augment(x, y)

    nc = _build(gamma_f, N_PER, N_Y, MODE)

    if MODE == "bf16x3":
        uts = _split_bf16(ut)
        vts = _split_bf16(vt)
    else:
        uts, vts = [ut], [vt]

    in_maps = []
    for c in range(N_CORES):
        m = {}
        for i, u in enumerate(uts):
            m[f"ut{i}"] = np.ascontiguousarray(u[:, c * N_PER : (c + 1) * N_PER])
        for i, v in enumerate(vts):
            m[f"vt{i}"] = v
        in_maps.append(m)

    res = run_bass_kernel_spmd(nc, in_maps, core_ids=list(range(N_CORES)))
    LAST_RESULTS = res
    return np.concatenate([res.results[c]["out"] for c in range(N_CORES)], axis=0)


# revision 4
# speedup vs baseline: 1.2858x; 1.2858x over previous
"""RBF (Gaussian) kernel matrix on 8 TRN2 NeuronCores.

out[i, j] = exp(-gamma * ||x_i - y_j||^2),  x: [8192, 64], y: [8192, 64].

Strategy: shard rows of x across 8 cores (each computes a [1024, 8192]
tile), replicate y.  The squared distance is produced directly by matmul
via augmented vectors:

    u_i = [-2*x_i, |x_i|^2, 1]   (K = 66)
    v_j = [   y_j,       1, |y_j|^2]

so  u_i . v_j = |x_i|^2 + |y_j|^2 - 2 x_i.y_j = dist2[i, j].  PSUM then
holds dist2 directly and one ScalarE activation computes
exp(-gamma * dist2) per [128, 512] tile — no vector-engine work at all.
Output is staged into [128, 8192] SBUF strips so each store is a single
4 MB DMA (near peak HBM write bandwidth).

MODE selects the matmul precision strategy:
  "f32"    — native fp32 matmul (4 cycles/row on the PE).
  "f32r"   — single-pass fp32 (tf32-like, 1 cycle/row), reduced precision.
  "bf16x3" — split each operand a = hi + lo in bf16 and accumulate
             hi*hi + hi*lo + lo*hi in PSUM (3 bf16 matmuls, ~18-bit
             effective mantissa, 3 cycles/row).
"""

import numpy as np

N_X, N_Y, D = 8192, 8192, 64
N_CORES = 8
N_PER = N_X // N_CORES  # rows of x per core
K_AUG = D + 2  # 66

MODE = "bf16x3"

# Filled by kernel() with the BassKernelResults of the last run
# (test.py reads exec_time_ns from here when BASS_TRACE=1).
LAST_RESULTS = None

_BUILD_CACHE = {}


def _build(gamma: float, n_per: int, m_tot: int, mode: str):
    """Build + compile the single-core Bass program (same on all cores)."""
    import concourse.bacc as bacc
    import concourse.mybir as mybir
    import concourse.tile as tile

    key = (gamma, n_per, m_tot, mode)
    if key in _BUILD_CACHE:
        return _BUILD_CACHE[key]

    dt = mybir.dt
    in_dt = {"f32": dt.float32, "f32r": dt.float32r, "bf16x3": dt.bfloat16}[mode]
    nsplit = 2 if mode == "bf16x3" else 1

    nc = bacc.Bacc("TRN2", target_bir_lowering=False, debug=False)
    ut_d = [
        nc.dram_tensor(f"ut{i}", [K_AUG, n_per], in_dt, kind="ExternalInput").ap()
        for i in range(nsplit)
    ]
    vt_d = [
        nc.dram_tensor(f"vt{i}", [K_AUG, m_tot], in_dt, kind="ExternalInput").ap()
        for i in range(nsplit)
    ]
    out_d = nc.dram_tensor("out", [n_per, m_tot], dt.float32, kind="ExternalOutput").ap()

    MB = n_per // 128  # M-blocks (output partition tiles)
    CHUNK = 2048  # ACT granularity: 4 PSUM banks per activation op
    NCHUNK = m_tot // CHUNK
    JB = CHUNK // 512  # matmuls (PSUM banks) per chunk

    with tile.TileContext(nc) as tc:
        with (
            tc.tile_pool(name="const", bufs=1) as cpool,
            tc.tile_pool(name="psum", bufs=2, space="PSUM") as psum_pool,
            tc.tile_pool(name="strip", bufs=2) as strip_pool,
        ):
            ut_s = []
            vt_s = []
            for i in range(nsplit):
                u = cpool.tile([K_AUG, n_per], in_dt, tag=f"ut{i}")
                nc.sync.dma_start(u[:], ut_d[i][:])
                ut_s.append(u)
                v = cpool.tile([K_AUG, m_tot], in_dt, tag=f"vt{i}")
                nc.sync.dma_start(v[:], vt_d[i][:])
                vt_s.append(v)

            for m in range(MB):
                strip = strip_pool.tile([128, m_tot], dt.float32)
                msl = slice(m * 128, (m + 1) * 128)
                for c in range(NCHUNK):
                    csl = slice(c * CHUNK, (c + 1) * CHUNK)
                    ps = psum_pool.tile([128, CHUNK], dt.float32)
                    # one matmul (or one split-accumulation group) per PSUM bank
                    if nsplit == 1:
                        for j in range(JB):
                            jsl = slice(j * 512, (j + 1) * 512)
                            vsl = slice(c * CHUNK + j * 512, c * CHUNK + (j + 1) * 512)
                            nc.tensor.matmul(
                                ps[:, jsl], ut_s[0][:, msl], vt_s[0][:, vsl]
                            )
                    else:
                        # hi*hi, hi*lo (same weights), then lo*hi — grouped by
                        # weights so the stationary operand reloads rarely
                        for wi, (uu, vv, st, sp) in enumerate(
                            ((0, 0, True, False), (0, 1, False, False), (1, 0, False, True))
                        ):
                            for j in range(JB):
                                jsl = slice(j * 512, (j + 1) * 512)
                                vsl = slice(
                                    c * CHUNK + j * 512, c * CHUNK + (j + 1) * 512
                                )
                                nc.tensor.matmul(
                                    ps[:, jsl],
                                    ut_s[uu][:, msl],
                                    vt_s[vv][:, vsl],
                                    start=st,
                                    stop=sp,
                                )
                    nc.scalar.activation(
                        strip[:, csl],
                        ps[:],
                        mybir.ActivationFunctionType.Exp,
                        scale=-gamma,
                    )
                nc.sync.dma_start(out_d[msl, :], strip[:])

    nc.compile()
    _BUILD_CACHE[key] = nc
    return nc


def _augment(x: np.ndarray, y: np.ndarray):
    """Host-side prep: build transposed augmented operands (O(N*D) work)."""
    x = np.asarray(x, dtype=np.float32)
    y = np.asarray(y, dtype=np.float32)
    x2 = np.einsum("nd,nd->n", x, x).astype(np.float32)
    y2 = np.einsum("nd,nd->n", y, y).astype(np.float32)

    ut = np.empty((K_AUG, x.shape[0]), dtype=np.float32)
    ut[:D] = (-2.0 * x).T
    ut[D] = x2
    ut[D + 1] = 1.0

    vt = np.empty((K_AUG, y.shape[0]), dtype=np.float32)
    vt[:D] = y.T
    vt[D] = 1.0
    vt[D + 1] = y2
    return ut, vt


def _split_bf16(a32: np.ndarray):
    import ml_dtypes

    hi = a32.astype(ml_dtypes.bfloat16)
    lo = (a32 - hi.astype(np.float32)).astype(ml_dtypes.bfloat16)
    return [hi, lo]


def kernel(x: np.ndarray, y: np.ndarray, gamma: np.ndarray) -> np.ndarray:
    global LAST_RESULTS
    from concourse.bass_utils import run_bass_kernel_spmd

    gamma_f = float(np.asarray(gamma).reshape(()))
    ut, vt = _augment(x, y)

    nc = _build(gamma_f, N_PER, N_Y, MODE)

    if MODE == "bf16x3":
        uts = _split_bf16(ut)
        vts = _split_bf16(vt)
    else:
        uts, vts = [ut], [vt]

    in_maps = []
    for c in range(N_CORES):
        m = {}
        for i, u in enumerate(uts):
            m[f"ut{i}"] = np.ascontiguousarray(u[:, c * N_PER : (c + 1) * N_PER])
        for i, v in enumerate(vts):
            m[f"vt{i}"] = v
        in_maps.append(m)

    res = run_bass_kernel_spmd(nc, in_maps, core_ids=list(range(N_CORES)))
    LAST_RESULTS = res
    return np.concatenate([res.results[c]["out"] for c in range(N_CORES)], axis=0)


# revision 5
# speedup vs baseline: 1.9053x; 1.4818x over previous
"""RBF (Gaussian) kernel matrix on 8 TRN2 NeuronCores.

out[i, j] = exp(-gamma * ||x_i - y_j||^2),  x: [8192, 64], y: [8192, 64].

Strategy: shard rows of x across 8 cores (each computes a [1024, 8192]
tile), replicate y.  The squared distance is produced directly by matmul
via augmented vectors:

    u_i = [-2*x_i, |x_i|^2, 1]   (K = 66)
    v_j = [   y_j,       1, |y_j|^2]

so  u_i . v_j = |x_i|^2 + |y_j|^2 - 2 x_i.y_j = dist2[i, j].  PSUM then
holds dist2 directly and one ScalarE activation computes
exp(-gamma * dist2) per [128, 512] tile — no vector-engine work at all.
Output is staged into [128, 8192] SBUF strips so each store is a single
4 MB DMA (near peak HBM write bandwidth).

MODE selects the matmul precision strategy:
  "f32"    — native fp32 matmul (4 cycles/row on the PE).
  "f32r"   — single-pass fp32 (tf32-like, 1 cycle/row), reduced precision.
  "bf16x3" — split each operand a = hi + lo in bf16 and accumulate
             hi*hi + hi*lo + lo*hi in PSUM (3 bf16 matmuls, ~18-bit
             effective mantissa, 3 cycles/row).
"""

import numpy as np

N_X, N_Y, D = 8192, 8192, 64
N_CORES = 8
N_PER = N_X // N_CORES  # rows of x per core
K_AUG = D + 2  # 66

MODE = "f32r"

# Filled by kernel() with the BassKernelResults of the last run
# (test.py reads exec_time_ns from here when BASS_TRACE=1).
LAST_RESULTS = None

_BUILD_CACHE = {}


def _build(gamma: float, n_per: int, m_tot: int, mode: str):
    """Build + compile the single-core Bass program (same on all cores)."""
    import concourse.bacc as bacc
    import concourse.mybir as mybir
    import concourse.tile as tile

    key = (gamma, n_per, m_tot, mode)
    if key in _BUILD_CACHE:
        return _BUILD_CACHE[key]

    dt = mybir.dt
    in_dt = {"f32": dt.float32, "f32r": dt.float32r, "bf16x3": dt.bfloat16}[mode]
    nsplit = 2 if mode == "bf16x3" else 1

    nc = bacc.Bacc("TRN2", target_bir_lowering=False, debug=False)
    ut_d = [
        nc.dram_tensor(f"ut{i}", [K_AUG, n_per], in_dt, kind="ExternalInput").ap()
        for i in range(nsplit)
    ]
    vt_d = [
        nc.dram_tensor(f"vt{i}", [K_AUG, m_tot], in_dt, kind="ExternalInput").ap()
        for i in range(nsplit)
    ]
    out_d = nc.dram_tensor("out", [n_per, m_tot], dt.float32, kind="ExternalOutput").ap()

    MB = n_per // 128  # M-blocks (output partition tiles)
    CHUNK = 2048  # ACT granularity: 4 PSUM banks per activation op
    NCHUNK = m_tot // CHUNK
    JB = CHUNK // 512  # matmuls (PSUM banks) per chunk

    with tile.TileContext(nc) as tc:
        with (
            tc.tile_pool(name="const", bufs=1) as cpool,
            tc.tile_pool(name="psum", bufs=2, space="PSUM") as psum_pool,
            tc.tile_pool(name="strip", bufs=2) as strip_pool,
        ):
            ut_s = []
            vt_s = []
            for i in range(nsplit):
                u = cpool.tile([K_AUG, n_per], in_dt, tag=f"ut{i}")
                nc.sync.dma_start(u[:], ut_d[i][:])
                ut_s.append(u)
                v = cpool.tile([K_AUG, m_tot], in_dt, tag=f"vt{i}")
                nc.sync.dma_start(v[:], vt_d[i][:])
                vt_s.append(v)

            for m in range(MB):
                strip = strip_pool.tile([128, m_tot], dt.float32)
                msl = slice(m * 128, (m + 1) * 128)
                for c in range(NCHUNK):
                    csl = slice(c * CHUNK, (c + 1) * CHUNK)
                    ps = psum_pool.tile([128, CHUNK], dt.float32)
                    # one matmul (or one split-accumulation group) per PSUM bank
                    if nsplit == 1:
                        for j in range(JB):
                            jsl = slice(j * 512, (j + 1) * 512)
                            vsl = slice(c * CHUNK + j * 512, c * CHUNK + (j + 1) * 512)
                            nc.tensor.matmul(
                                ps[:, jsl], ut_s[0][:, msl], vt_s[0][:, vsl]
                            )
                    else:
                        # hi*hi, hi*lo (same weights), then lo*hi — grouped by
                        # weights so the stationary operand reloads rarely
                        for wi, (uu, vv, st, sp) in enumerate(
                            ((0, 0, True, False), (0, 1, False, False), (1, 0, False, True))
                        ):
                            for j in range(JB):
                                jsl = slice(j * 512, (j + 1) * 512)
                                vsl = slice(
                                    c * CHUNK + j * 512, c * CHUNK + (j + 1) * 512
                                )
                                nc.tensor.matmul(
                                    ps[:, jsl],
                                    ut_s[uu][:, msl],
                                    vt_s[vv][:, vsl],
                                    start=st,
                                    stop=sp,
                                )
                    nc.scalar.activation(
                        strip[:, csl],
                        ps[:],
                        mybir.ActivationFunctionType.Exp,
                        scale=-gamma,
                    )
                nc.sync.dma_start(out_d[msl, :], strip[:])

    nc.compile()
    _BUILD_CACHE[key] = nc
    return nc


def _augment(x: np.ndarray, y: np.ndarray):
    """Host-side prep: build transposed augmented operands (O(N*D) work)."""
    x = np.asarray(x, dtype=np.float32)
    y = np.asarray(y, dtype=np.float32)
    x2 = np.einsum("nd,nd->n", x, x).astype(np.float32)
    y2 = np.einsum("nd,nd->n", y, y).astype(np.float32)

    ut = np.empty((K_AUG, x.shape[0]), dtype=np.float32)
    ut[:D] = (-2.0 * x).T
    ut[D] = x2
    ut[D + 1] = 1.0

    vt = np.empty((K_AUG, y.shape[0]), dtype=np.float32)
    vt[:D] = y.T
    vt[D] = 1.0
    vt[D + 1] = y2
    return ut, vt


def _split_bf16(a32: np.ndarray):
    import ml_dtypes

    hi = a32.astype(ml_dtypes.bfloat16)
    lo = (a32 - hi.astype(np.float32)).astype(ml_dtypes.bfloat16)
    return [hi, lo]


def kernel(x: np.ndarray, y: np.ndarray, gamma: np.ndarray) -> np.ndarray:
    global LAST_RESULTS
    from concourse.bass_utils import run_bass_kernel_spmd

    gamma_f = float(np.asarray(gamma).reshape(()))
    ut, vt = _augment(x, y)

    nc = _build(gamma_f, N_PER, N_Y, MODE)

    if MODE == "bf16x3":
        uts = _split_bf16(ut)
        vts = _split_bf16(vt)
    else:
        uts, vts = [ut], [vt]

    in_maps = []
    for c in range(N_CORES):
        m = {}
        for i, u in enumerate(uts):
            m[f"ut{i}"] = np.ascontiguousarray(u[:, c * N_PER : (c + 1) * N_PER])
        for i, v in enumerate(vts):
            m[f"vt{i}"] = v
        in_maps.append(m)

    res = run_bass_kernel_spmd(nc, in_maps, core_ids=list(range(N_CORES)))
    LAST_RESULTS = res
    return np.concatenate([res.results[c]["out"] for c in range(N_CORES)], axis=0)
